# revision 60
# baseline (speedup 1.0000x reference)
"""Distributed Trainium2 (8 NeuronCores) kernel for masked multi-head attention
+ output projection (nn_Attention_60790967107825).

Head-parallel attention over a mask-COMPACTED key *and query* set,
row-parallel projection, one AllToAll per head-half:

  - The mask applies to both queries and keys (m2 = m_i & m_j). Masked
    queries see an all-masked row -> uniform attention over ALL N keys;
    that output is a single per-batch constant row computed on the HOST
    (mean(V) @ W^T + b). The device therefore computes attention ONLY for
    the ~50% unmasked queries, against the ~50% unmasked keys (masked keys
    contribute exp(-inf)=0 exactly): ~4x less matmul+exp work than dense.
  - Each core owns 2 of the 16 heads x 4 batches = 8 (b,h) pairs. q/k are
    fed pre-transposed [D, nq] so the S^T = K Q^T matmul needs no on-device
    transposes and runs with a 64-deep contraction (no zero padding).
  - A ones-column appended to V yields the softmax denominators as row 64
    of the PV accumulation for free; pad slots carry k=0/v=0/ones=0 so they
    contribute nothing.
  - The PE stream is software-pipelined one step ahead (S(i+1) is emitted
    before PV(i)) so the tensor engine never head-of-line blocks on the
    activation engine's exp, keeping it at the full 2.4 GHz p-state.
  - Numerators are scaled by 1/Z via DVE fast reciprocal on the PSUM Z row,
    a GPSIMD partition broadcast, and one fused DVE multiply from PSUM.
  - Two AllToAlls (one per head-half) redistribute head-major outputs to
    row-major shards; the first hides under compute, warm-up matmuls bridge
    the second so the projection starts at full clock. Projection output
    rows go PSUM -> DRAM directly; bias is added on the host.
"""

import os
import sys

import numpy as np

for _p in ("/opt/trn_rl_repo", "/root/.axon_site/_ro/trn_rl_repo"):
    if os.path.isdir(_p) and _p not in sys.path:
        sys.path.insert(0, _p)

import ml_dtypes  # noqa: E402
import concourse.bass as bass  # noqa: E402,F401
import concourse.mybir as mybir  # noqa: E402
import concourse.tile as tile  # noqa: E402
from concourse import bacc  # noqa: E402
from concourse.bass_utils import run_bass_kernel_spmd  # noqa: E402

B, H, N, D = 4, 16, 2048, 64
DIM = H * D
P = 128
NCORES = 8
HPC = H // NCORES          # heads per core
PAIRS = B * HPC            # (b, h_local) pairs per core
SCALE = float(D) ** -0.5
CT = DIM // P              # 8 contraction tiles in the projection
CW = 512                   # max query-chunk width (one PSUM bank fp32)

bf16 = mybir.dt.bfloat16
f32 = mybir.dt.float32
npbf = ml_dtypes.bfloat16

_CACHE = {}


def chunk_widths(np_b):
    """Split np_b (multiple of 128) into full CW chunks plus a remainder."""
    ws = []
    r = np_b
    while r > 0:
        w = min(CW, r)
        ws.append(w)
        r -= w
    return ws


def jt_groups(jtk):
    """Pair key tiles so each exp instruction covers two of them."""
    gs = [list(range(j, min(j + 2, jtk))) for j in range(0, jtk, 2)]
    return gs


def build_graph(npb):
    npmax = max(npb)
    TQ = sum(npb)
    RBq = TQ // NCORES          # projection rows owned per core
    NRT = -(-RBq // P)          # projection row tiles (last may be partial)
    G = [sum(npb[:b]) for b in range(B)]  # global row offset per batch

    nc = bacc.Bacc("TRN2", num_devices=NCORES)

    qT = nc.dram_tensor("qT", [PAIRS, D, npmax], bf16, kind="ExternalInput")
    kT = nc.dram_tensor("kT", [PAIRS, D, npmax], bf16, kind="ExternalInput")
    vv = nc.dram_tensor("v", [PAIRS, npmax, D + 1], bf16, kind="ExternalInput")
    # W^T rows regrouped per head-half so each half's projection contracts
    # over full 128-deep tiles: wTs[hl][cp*128 + par*64 + j] =
    # wT[(4*cp + 2*par + hl)*64 + j]
    wTD = nc.dram_tensor("wTs", [HPC, 4 * P, DIM], bf16, kind="ExternalInput")
    outD = nc.dram_tensor("out", [RBq, DIM], f32, kind="ExternalOutput")

    RBH = RBq // 2  # half-width A2A buffers (two pipelined collectives)

    def dest_splits(g0, w):
        """Split global row range [g0, g0+w) by owning core and A2A half."""
        res = []
        g = g0
        while g < g0 + w:
            r = g // RBq
            l = g - r * RBq
            half = l // RBH
            hi = min(r * RBq + (half + 1) * RBH, g0 + w)
            res.append((r, half, l - half * RBH, g - g0, hi - g0))
            g = hi
        return res

    with tile.TileContext(nc, num_cores=NCORES) as tc:
        with tc.tile_pool(name="dram", bufs=1, space="DRAM") as dramp:
            a2a_in = [
                [
                    dramp.tile([NCORES, D, RBH], bf16, name=f"a2a_in{h}_{x}")
                    for x in range(2)
                ]
                for h in range(HPC)
            ]
            a2a_out = [
                [
                    dramp.tile([NCORES, D, RBH], bf16, name=f"a2a_out{h}_{x}")
                    for x in range(2)
                ]
                for h in range(HPC)
            ]

            with tc.tile_pool(name="constp", bufs=1) as constp:
                wt_sb = constp.tile([P, HPC, 4, DIM], bf16, name="wt_sb")
                gat = [
                    constp.tile([P, 4, RBq], bf16, name=f"gat{h}")
                    for h in range(HPC)
                ]
                osb0 = constp.tile([P, 1 + (RBq - 1) // P, DIM], f32, name="osb0")
                warmw = constp.tile([P, CW], bf16, name="warmw")

                with (
                    tc.tile_pool(name="qkp", bufs=6) as qkp,
                    tc.tile_pool(name="vpool", bufs=6) as vp,
                    tc.tile_pool(name="ptp", bufs=4) as ptp,
                    tc.tile_pool(name="zp", bufs=2) as zp,
                    tc.tile_pool(name="zmp", bufs=2) as zmp,
                    tc.tile_pool(name="finp", bufs=2) as finp,
                    tc.tile_pool(name="psS", bufs=2, space="PSUM") as psS,
                    tc.tile_pool(name="psO", bufs=3, space="PSUM") as psO,
                    tc.tile_pool(name="psF", bufs=1, space="PSUM") as psF,
                ):
                    # dedicated filler target: filler matmuls never have any
                    # dependency (same-engine WAW only), so they can spin the
                    # PE through exp bubbles and hold the p-state ramped
                    fps = psF.tile([P, CW], f32, name="fps")

                    def filler(cols):
                        return nc.tensor.matmul(
                            fps[:, :cols],
                            lhsT=warmw[:, 0:P],
                            rhs=warmw[:, :cols],
                            start=True,
                            stop=True,
                            skip_group_check=True,
                        )

                    # startup warm-ups: ramp the PE p-state while the first
                    # pair's DMAs are in flight (no data dependencies)
                    nc.vector.memset(warmw[:], 0.0)
                    for i in range(4):
                        filler(CW)

                    last_pv = None
                    pending = []     # deferred PVs (2-step software pipeline)
                    evac_q = []      # chunks awaiting evacuation, in order
                    cc_q = None      # head-half awaiting collective emission

                    def emit_evac(ctx):
                        o_t, w, hl, g0 = ctx
                        # custom-DVE ops require base partition 0: stage the
                        # PSUM Z row (partition 64) into SBUF partition 0
                        zc = zp.tile([1, CW], f32, tag="zc")
                        nc.vector.tensor_copy(zc[:, :w], o_t[D : D + 1, :w])
                        zr = zp.tile([1, CW], f32, tag="zr")
                        nc.vector.reciprocal_approx_fast(zr[:, :w], zc[:, :w])
                        zm = zmp.tile([D, CW], f32, tag="zm")
                        nc.gpsimd.partition_broadcast(
                            zm[:, :w], zr[:, :w], channels=D
                        )
                        fin = finp.tile([D, CW], bf16, tag="fin")
                        nc.vector.tensor_tensor(
                            fin[:, :w], o_t[:D, :w], zm[:, :w],
                            mybir.AluOpType.mult,
                        )
                        for (r, half, l0, c0, c1) in dest_splits(g0, w):
                            # gpsimd SWDGE queue: keeps the fins off the SP
                            # HWDGE rings so they can never interleave with
                            # (and block) the pair loads
                            nc.gpsimd.dma_start(
                                a2a_in[hl][half][r, :, l0 : l0 + (c1 - c0)],
                                fin[:, c0:c1],
                            )

                    def flush_one():
                        nonlocal cc_q, last_pv
                        if not pending:
                            return
                        o_t, vt_t, pt_t, w, grp, jtk = pending.pop(0)
                        for slot, jt in enumerate(grp):
                            last_pv = nc.tensor.matmul(
                                o_t[: D + 1, :w],
                                lhsT=vt_t[:, jt, :],
                                rhs=pt_t[:, slot, :w],
                                start=(jt == 0),
                                stop=(jt == jtk - 1),
                            )
                        if grp[-1] == jtk - 1 and evac_q:
                            emit_evac(evac_q.pop(0))
                            if cc_q is not None:
                                hl = cc_q
                                cc_q = None
                                for x in range(2):
                                    nc.gpsimd.collective_compute(
                                        "AllToAll",
                                        mybir.AluOpType.bypass,
                                        replica_groups=[list(range(NCORES))],
                                        ins=[a2a_in[hl][x].opt()],
                                        outs=[a2a_out[hl][x].opt()],
                                    )

                    tiles = {}

                    def load_pair(hl, b, first):
                        pr = b * HPC + hl
                        np_b = npb[b]
                        jtk = np_b // P
                        ws = chunk_widths(np_b)
                        qt = qkp.tile([P, npmax], bf16, tag="qt", name=f"qt{pr}")
                        kt = qkp.tile([P, npmax], bf16, tag="kt", name=f"kt{pr}")
                        # pad rows D:P with zeros: full 128-deep tile
                        # geometry keeps the PE column rate at 2x
                        nc.gpsimd.memset(qt[D:, :np_b], 0.0)
                        nc.gpsimd.memset(kt[D:, :np_b], 0.0)
                        ksp = (0, P, 4 * P, np_b) if first else (0, np_b)
                        for lo, hi in zip(ksp[:-1], ksp[1:]):
                            if lo < hi:
                                nc.sync.dma_start(kt[:D, lo:hi], kT[pr, :, lo:hi])
                        qsp = (0, ws[0], np_b) if first else (0, np_b)
                        for lo, hi in zip(qsp[:-1], qsp[1:]):
                            if lo < hi:
                                nc.sync.dma_start(qt[:D, lo:hi], qT[pr, :, lo:hi])
                        vt = vp.tile(
                            [P, jtk, D + 1], bf16, tag="vt", name=f"vt{pr}"
                        )
                        vsp = ((0, max(jtk // 2, 1)), (max(jtk // 2, 1), jtk)) if (
                            first
                        ) else ((0, jtk),)
                        for lo, hi in vsp:
                            if lo < hi:
                                nc.sync.dma_start(
                                    vt[:, lo:hi, :],
                                    vv[pr, lo * P : hi * P, :]
                                    .rearrange("(t pp) d -> pp t d", pp=P),
                                )
                        tiles[(hl, b)] = (qt, kt, vt)

                    flat = [(hl, b) for hl in range(HPC) for b in range(B)]
                    load_pair(*flat[0], True)
                    for h2 in range(HPC):
                        nc.sync.dma_start(
                            wt_sb[:, h2, :, :],
                            wTD[h2].rearrange("(c p) n -> p c n", p=P),
                        )
                    for nb in flat[1:4]:
                        load_pair(*nb, False)

                    for idx, (hl, b) in enumerate(flat):
                        if idx + 4 < len(flat):
                            load_pair(*flat[idx + 4], False)
                        pr = b * HPC + hl
                        np_b = npb[b]
                        jtk = np_b // P
                        ws = chunk_widths(np_b)
                        qt, kt, vt = tiles[(hl, b)]

                        if True:
                            off = 0
                            for ci, w in enumerate(ws):
                                o_t = psO.tile(
                                    [P, CW], f32, tag="ops",
                                    name=f"o{pr}_{ci}",
                                )
                                for gi, grp in enumerate(jt_groups(jtk)):
                                    s_t = psS.tile(
                                        [P, 2, CW], f32, tag="sps",
                                        name=f"s{pr}_{ci}_{gi}",
                                    )
                                    for slot, jt in enumerate(grp):
                                        nc.tensor.matmul(
                                            s_t[:, slot, :w],
                                            lhsT=kt[:, jt * P : (jt + 1) * P],
                                            rhs=qt[:, off : off + w],
                                            start=True,
                                            stop=True,
                                        )
                                    pt = ptp.tile(
                                        [P, 2, CW], bf16, tag="pt",
                                        name=f"p{pr}_{ci}_{gi}",
                                    )
                                    ng = len(grp)
                                    nc.scalar.activation(
                                        pt[:, 0:ng, :w],
                                        s_t[:, 0:ng, :w],
                                        mybir.ActivationFunctionType.Exp,
                                        scale=SCALE,
                                    )
                                    if len(pending) >= 2:
                                        flush_one()
                                    pending.append((o_t, vt, pt, w, grp, jtk))
                                    if grp[-1] == jtk - 1:
                                        evac_q.append((o_t, w, hl, G[b] + off))
                                off += w
                        if b == B - 1:
                            # exchange this head-half once its last chunk's
                            # PV + evacuation are flushed (a couple of steps
                            # into the next pair for hl=0)
                            cc_q = hl
                    while pending:
                        flush_one()

                    # bridge warm-ups: keep the PE clock ramped through the
                    # second A2A + gather window (psS-pool target so they
                    # don't WAR-stall on the final evacuation's PSUM reads)
                    def pin(mm, after, why):
                        tile.add_dep_helper(
                            mm.ins, after.ins, sync=False, reason=why
                        )
                        return mm

                with (
                    tc.tile_pool(name="outp", bufs=2) as outp,
                    tc.tile_pool(name="psP", bufs=2, space="PSUM") as psP,
                    tc.tile_pool(name="psW", bufs=1, space="PSUM") as psW,
                ):
                    wbA = psW.tile([P, 3, CW], f32, name="wbA")
                    def gather_half(h):
                        # pack src pairs (2cp, 2cp+1) into 128-deep tiles
                        for x in range(2):
                            for par in range(2):
                                nc.sync.dma_start(
                                    gat[h][
                                        par * D : (par + 1) * D,
                                        :,
                                        x * RBH : (x + 1) * RBH,
                                    ],
                                    a2a_out[h][x][par::2]
                                    .rearrange("c d l -> d c l"),
                                )

                    def proj_half(h, emit_out):
                        # interleave two row tiles' matmuls so consecutive
                        # PE instructions hit different PSUM banks
                        for rtp in range(0, NRT, 2):
                            rts = [rt for rt in (rtp, rtp + 1) if rt < NRT]
                            pt_ps = {
                                rt: psP.tile(
                                    [P, DIM], f32, tag="prps",
                                    name=f"pr{h}_{rt}",
                                )
                                for rt in rts
                            }
                            for cp in range(4):
                                for n0 in range(0, DIM, 512):
                                    for rt in rts:
                                        rows = min(P, RBq - rt * P)
                                        pin(
                                            nc.tensor.matmul(
                                                pt_ps[rt][:rows, n0 : n0 + 512],
                                                lhsT=gat[h][
                                                    :, cp, rt * P : rt * P + rows
                                                ],
                                                rhs=wt_sb[:, h, cp, n0 : n0 + 512],
                                                start=(cp == 0),
                                                stop=(cp == 3),
                                            ),
                                            last_pv,
                                            "projection follows attention",
                                        )
                            for rt in rts:
                                emit_out(rt, pt_ps[rt])

                    # hl=0's A2A completed mid-attention: its half of the
                    # projection runs DURING the second collective's window
                    gather_half(0)

                    def out0(rt, ps):
                        rows = min(P, RBq - rt * P)
                        nc.vector.tensor_copy(osb0[:rows, rt, :], ps[:rows, :])

                    proj_half(0, out0)

                    # bridge warm-ups: rotate targets across 3 PSUM banks so
                    # the drains pipeline
                    wtgts = [wbA[:, 0, :], wbA[:, 1, :], wbA[:, 2, :]]
                    last_warm = last_pv
                    for i in range(40):
                        last_warm = pin(
                            nc.tensor.matmul(
                                wtgts[i % 3],
                                lhsT=warmw[:, 0:P],
                                rhs=warmw[:],
                                start=True,
                                stop=True,
                                skip_group_check=True,
                            ),
                            last_pv,
                            "warmups bridge the A2A window",
                        )

                    gather_half(1)

                    def out1(rt, ps):
                        rows = min(P, RBq - rt * P)
                        osb = outp.tile([P, DIM], f32, tag="osb", name=f"ob{rt}")
                        nc.vector.tensor_tensor(
                            osb[:rows, :],
                            osb0[:rows, rt, :],
                            ps[:rows, :],
                            mybir.AluOpType.add,
                        )
                        nc.sync.dma_start(
                            outD[rt * P : rt * P + rows, :], osb[:rows, :]
                        )

                    proj_half(1, out1)

    nc.compile()
    return nc


def _get_nc(npb):
    key = f"nc{npb}"
    if key not in _CACHE:
        _CACHE[key] = build_graph(npb)
    return _CACHE[key]


def key_budget(mask):
    """Per-batch compacted row counts (unmasked incl. CLS), padded to 128."""
    counts = 1 + np.asarray(mask).astype(bool).sum(axis=1)
    return tuple(
        min(max(int(-(-int(c) // P) * P), P), N) for c in counts
    )


def make_in_maps(q, k, v, mask, W_out, b_out, npb):
    npmax = max(npb)
    q16 = np.asarray(q).astype(npbf)
    k16 = np.asarray(k).astype(npbf)
    v16 = np.asarray(v).astype(npbf)
    m_full = np.concatenate(
        [np.ones((B, 1), dtype=bool), np.asarray(mask).astype(bool)], axis=1
    )  # [B, N]

    qTall = np.zeros((B, H, D, npmax), dtype=npbf)
    kTall = np.zeros((B, H, D, npmax), dtype=npbf)
    vall = np.zeros((B, H, npmax, D + 1), dtype=npbf)
    for b in range(B):
        idx = np.flatnonzero(m_full[b])
        c = len(idx)
        qTall[b, :, :, :c] = q16[b][:, idx, :].transpose(0, 2, 1)
        kTall[b, :, :, :c] = k16[b][:, idx, :].transpose(0, 2, 1)
        vall[b, :, :c, :D] = v16[b][:, idx, :]
        vall[b, :, :c, D] = 1.0

    wT16 = np.ascontiguousarray(np.asarray(W_out).T).astype(npbf)
    # regroup W^T rows per head-half (see build_graph's wTs comment)
    wTs = np.empty((HPC, 4 * P, DIM), dtype=npbf)
    for hl in range(HPC):
        for cp in range(4):
            for par in range(2):
                h = 4 * cp + 2 * par + hl
                wTs[hl, cp * P + par * D : cp * P + (par + 1) * D] = wT16[
                    h * D : (h + 1) * D
                ]

    in_maps = []
    for c in range(NCORES):
        heads = slice(HPC * c, HPC * (c + 1))
        in_maps.append(
            {
                "qT": np.ascontiguousarray(
                    qTall[:, heads].reshape(PAIRS, D, npmax)
                ),
                "kT": np.ascontiguousarray(
                    kTall[:, heads].reshape(PAIRS, D, npmax)
                ),
                "v": np.ascontiguousarray(
                    vall[:, heads].reshape(PAIRS, npmax, D + 1)
                ),
                "wTs": wTs,
            }
        )
    return in_maps


def run(q, k, v, mask, W_out, b_out, trace=False, **spmd_kwargs):
    npb = key_budget(mask)
    nc = _get_nc(npb)
    in_maps = make_in_maps(q, k, v, mask, W_out, b_out, npb)
    res = run_bass_kernel_spmd(
        nc, in_maps, core_ids=list(range(NCORES)), trace=trace, **spmd_kwargs
    )
    proj = np.concatenate(
        [np.asarray(res.results[c]["out"]) for c in range(NCORES)], axis=0
    )  # [TQ, DIM]

    m_full = np.concatenate(
        [np.ones((B, 1), dtype=bool), np.asarray(mask).astype(bool)], axis=1
    )
    W32 = np.asarray(W_out, dtype=np.float32)
    b32 = np.asarray(b_out, dtype=np.float32)
    v32 = np.asarray(v, dtype=np.float32)
    full = np.empty((B, N, DIM), dtype=np.float32)
    g0 = 0
    for b in range(B):
        idx = np.flatnonzero(m_full[b])
        full[b, idx] = proj[g0 : g0 + len(idx)] + b32
        # masked queries: uniform attention over ALL N keys
        vmean = v32[b].transpose(1, 0, 2).reshape(N, DIM).mean(axis=0)
        full[b, ~m_full[b]] = vmean @ W32.T + b32
        g0 += npb[b]
    return full, res


def kernel(q, k, v, mask, W_out, b_out):
    out, _ = run(q, k, v, mask, W_out, b_out, trace=False)
    return out


# revision 62
# speedup vs baseline: 1.0522x; 1.0522x over previous
"""Distributed Trainium2 (8 NeuronCores) kernel for masked multi-head attention
+ output projection (nn_Attention_60790967107825).

Head-parallel attention over a mask-COMPACTED key *and query* set,
row-parallel projection, one AllToAll per head-half:

  - The mask applies to both queries and keys (m2 = m_i & m_j). Masked
    queries see an all-masked row -> uniform attention over ALL N keys;
    that output is a single per-batch constant row computed on the HOST
    (mean(V) @ W^T + b). The device therefore computes attention ONLY for
    the ~50% unmasked queries, against the ~50% unmasked keys (masked keys
    contribute exp(-inf)=0 exactly): ~4x less matmul+exp work than dense.
  - Each core owns 2 of the 16 heads x 4 batches = 8 (b,h) pairs. q/k are
    fed pre-transposed [D, nq] so the S^T = K Q^T matmul needs no on-device
    transposes and runs with a 64-deep contraction (no zero padding).
  - A ones-column appended to V yields the softmax denominators as row 64
    of the PV accumulation for free; pad slots carry k=0/v=0/ones=0 so they
    contribute nothing.
  - The PE stream is software-pipelined one step ahead (S(i+1) is emitted
    before PV(i)) so the tensor engine never head-of-line blocks on the
    activation engine's exp, keeping it at the full 2.4 GHz p-state.
  - Numerators are scaled by 1/Z via DVE fast reciprocal on the PSUM Z row,
    a GPSIMD partition broadcast, and one fused DVE multiply from PSUM.
  - Two AllToAlls (one per head-half) redistribute head-major outputs to
    row-major shards; the first hides under compute, warm-up matmuls bridge
    the second so the projection starts at full clock. Projection output
    rows go PSUM -> DRAM directly; bias is added on the host.
"""

import os
import sys

import numpy as np

for _p in ("/opt/trn_rl_repo", "/root/.axon_site/_ro/trn_rl_repo"):
    if os.path.isdir(_p) and _p not in sys.path:
        sys.path.insert(0, _p)

import ml_dtypes  # noqa: E402
import concourse.bass as bass  # noqa: E402,F401
import concourse.mybir as mybir  # noqa: E402
import concourse.tile as tile  # noqa: E402
from concourse import bacc  # noqa: E402
from concourse.bass_utils import run_bass_kernel_spmd  # noqa: E402

B, H, N, D = 4, 16, 2048, 64
DIM = H * D
P = 128
NCORES = 8
HPC = H // NCORES          # heads per core
PAIRS = B * HPC            # (b, h_local) pairs per core
SCALE = float(D) ** -0.5
CT = DIM // P              # 8 contraction tiles in the projection
CW = 512                   # max query-chunk width (one PSUM bank fp32)

bf16 = mybir.dt.bfloat16
f32 = mybir.dt.float32
npbf = ml_dtypes.bfloat16

_CACHE = {}


def chunk_widths(np_b):
    """Split np_b (multiple of 128) into full CW chunks plus a remainder."""
    ws = []
    r = np_b
    while r > 0:
        w = min(CW, r)
        ws.append(w)
        r -= w
    return ws


def jt_groups(jtk):
    """Pair key tiles so each exp instruction covers two of them."""
    gs = [list(range(j, min(j + 2, jtk))) for j in range(0, jtk, 2)]
    return gs


def build_graph(npb):
    npmax = max(npb)
    TQ = sum(npb)
    RBq = TQ // NCORES          # projection rows owned per core
    NRT = -(-RBq // P)          # projection row tiles (last may be partial)
    G = [sum(npb[:b]) for b in range(B)]  # global row offset per batch

    nc = bacc.Bacc("TRN2", num_devices=NCORES)

    qT = nc.dram_tensor("qT", [PAIRS, D, npmax], bf16, kind="ExternalInput")
    kT = nc.dram_tensor("kT", [PAIRS, D, npmax], bf16, kind="ExternalInput")
    vv = nc.dram_tensor("v", [PAIRS, npmax, D + 1], bf16, kind="ExternalInput")
    # W^T rows regrouped per head-half so each half's projection contracts
    # over full 128-deep tiles: wTs[hl][cp*128 + par*64 + j] =
    # wT[(4*cp + 2*par + hl)*64 + j]
    wTD = nc.dram_tensor("wTs", [HPC, 4 * P, DIM], bf16, kind="ExternalInput")
    outD = nc.dram_tensor("out", [RBq, DIM], f32, kind="ExternalOutput")

    RBH = RBq // 2  # half-width A2A buffers (two pipelined collectives)

    def dest_splits(g0, w):
        """Split global row range [g0, g0+w) by owning core and A2A half."""
        res = []
        g = g0
        while g < g0 + w:
            r = g // RBq
            l = g - r * RBq
            half = l // RBH
            hi = min(r * RBq + (half + 1) * RBH, g0 + w)
            res.append((r, half, l - half * RBH, g - g0, hi - g0))
            g = hi
        return res

    with tile.TileContext(nc, num_cores=NCORES) as tc:
        with tc.tile_pool(name="dram", bufs=1, space="DRAM") as dramp:
            a2a_in = [
                [
                    dramp.tile([NCORES, D, RBH], bf16, name=f"a2a_in{h}_{x}")
                    for x in range(2)
                ]
                for h in range(HPC)
            ]
            a2a_out = [
                [
                    dramp.tile([NCORES, D, RBH], bf16, name=f"a2a_out{h}_{x}")
                    for x in range(2)
                ]
                for h in range(HPC)
            ]

            with tc.tile_pool(name="constp", bufs=1) as constp:
                wt_sb = constp.tile([P, HPC, 4, DIM], bf16, name="wt_sb")
                gat = [
                    constp.tile([P, 4, RBq], bf16, name=f"gat{h}")
                    for h in range(HPC)
                ]
                osb0 = constp.tile([P, 1 + (RBq - 1) // P, DIM], f32, name="osb0")
                warmw = constp.tile([P, CW], bf16, name="warmw")

                with (
                    tc.tile_pool(name="qkp", bufs=6) as qkp,
                    tc.tile_pool(name="vpool", bufs=6) as vp,
                    tc.tile_pool(name="ptp", bufs=4) as ptp,
                    tc.tile_pool(name="zp", bufs=2) as zp,
                    tc.tile_pool(name="zmp", bufs=2) as zmp,
                    tc.tile_pool(name="finp", bufs=2) as finp,
                    tc.tile_pool(name="psS", bufs=3, space="PSUM") as psS,
                    tc.tile_pool(name="psO", bufs=2, space="PSUM") as psO,
                ):
                    # filler target: startup filler matmuls have no data
                    # dependency (same-engine WAW only), so they ramp the PE
                    # p-state while the first loads are in flight
                    fps = psS.tile([P, 2, CW], f32, tag="sps", name="fps")[:, 0, :]

                    def filler(cols):
                        return nc.tensor.matmul(
                            fps[:, :cols],
                            lhsT=warmw[:, 0:P],
                            rhs=warmw[:, :cols],
                            start=True,
                            stop=True,
                            skip_group_check=True,
                        )

                    # startup warm-ups: ramp the PE p-state while the first
                    # pair's DMAs are in flight (no data dependencies)
                    nc.vector.memset(warmw[:], 0.0)
                    for i in range(4):
                        filler(CW)

                    last_pv = None
                    pending = []     # deferred PVs (2-step software pipeline)
                    evac_q = []      # chunks awaiting evacuation, in order
                    cc_q = None      # head-half awaiting collective emission

                    def emit_evac(ctx):
                        o_t, w, hl, g0 = ctx
                        # custom-DVE ops require base partition 0: stage the
                        # PSUM Z row (partition 64) into SBUF partition 0
                        zc = zp.tile([1, CW], f32, tag="zc")
                        nc.vector.tensor_copy(zc[:, :w], o_t[D : D + 1, :w])
                        zr = zp.tile([1, CW], f32, tag="zr")
                        nc.vector.reciprocal_approx_fast(zr[:, :w], zc[:, :w])
                        zm = zmp.tile([D, CW], f32, tag="zm")
                        nc.gpsimd.partition_broadcast(
                            zm[:, :w], zr[:, :w], channels=D
                        )
                        fin = finp.tile([D, CW], bf16, tag="fin")
                        nc.vector.tensor_tensor(
                            fin[:, :w], o_t[:D, :w], zm[:, :w],
                            mybir.AluOpType.mult,
                        )
                        for (r, half, l0, c0, c1) in dest_splits(g0, w):
                            # gpsimd SWDGE queue: keeps the fins off the SP
                            # HWDGE rings so they can never interleave with
                            # (and block) the pair loads
                            nc.gpsimd.dma_start(
                                a2a_in[hl][half][r, :, l0 : l0 + (c1 - c0)],
                                fin[:, c0:c1],
                            )

                    def flush_one():
                        nonlocal cc_q, last_pv
                        if not pending:
                            return
                        o_t, vt_t, pt_t, w, grp, jtk = pending.pop(0)
                        for slot, jt in enumerate(grp):
                            last_pv = nc.tensor.matmul(
                                o_t[: D + 1, :w],
                                lhsT=vt_t[:, jt, :],
                                rhs=pt_t[:, slot, :w],
                                start=(jt == 0),
                                stop=(jt == jtk - 1),
                            )
                        if grp[-1] == jtk - 1 and evac_q:
                            emit_evac(evac_q.pop(0))
                            if cc_q is not None:
                                hl = cc_q
                                cc_q = None
                                for x in range(2):
                                    nc.gpsimd.collective_compute(
                                        "AllToAll",
                                        mybir.AluOpType.bypass,
                                        replica_groups=[list(range(NCORES))],
                                        ins=[a2a_in[hl][x].opt()],
                                        outs=[a2a_out[hl][x].opt()],
                                    )

                    tiles = {}

                    def load_pair(hl, b, first):
                        pr = b * HPC + hl
                        np_b = npb[b]
                        jtk = np_b // P
                        ws = chunk_widths(np_b)
                        qt = qkp.tile([P, npmax], bf16, tag="qt", name=f"qt{pr}")
                        kt = qkp.tile([P, npmax], bf16, tag="kt", name=f"kt{pr}")
                        # pad rows D:P with zeros: full 128-deep tile
                        # geometry keeps the PE column rate at 2x
                        nc.gpsimd.memset(qt[D:, :np_b], 0.0)
                        nc.gpsimd.memset(kt[D:, :np_b], 0.0)
                        ksp = (0, P, 4 * P, np_b) if first else (0, np_b)
                        for lo, hi in zip(ksp[:-1], ksp[1:]):
                            if lo < hi:
                                nc.sync.dma_start(kt[:D, lo:hi], kT[pr, :, lo:hi])
                        qsp = (0, ws[0], np_b) if first else (0, np_b)
                        for lo, hi in zip(qsp[:-1], qsp[1:]):
                            if lo < hi:
                                nc.sync.dma_start(qt[:D, lo:hi], qT[pr, :, lo:hi])
                        vt = vp.tile(
                            [P, jtk, D + 1], bf16, tag="vt", name=f"vt{pr}"
                        )
                        vsp = ((0, max(jtk // 2, 1)), (max(jtk // 2, 1), jtk)) if (
                            first
                        ) else ((0, jtk),)
                        for lo, hi in vsp:
                            if lo < hi:
                                nc.sync.dma_start(
                                    vt[:, lo:hi, :],
                                    vv[pr, lo * P : hi * P, :]
                                    .rearrange("(t pp) d -> pp t d", pp=P),
                                )
                        tiles[(hl, b)] = (qt, kt, vt)

                    flat = [(hl, b) for hl in range(HPC) for b in range(B)]
                    load_pair(*flat[0], True)
                    for nb in flat[1:4]:
                        load_pair(*nb, False)
                    # projection weights after the first four pairs' loads:
                    # 2MB that is not needed until the projection phase
                    for h2 in range(HPC):
                        nc.sync.dma_start(
                            wt_sb[:, h2, :, :],
                            wTD[h2].rearrange("(c p) n -> p c n", p=P),
                        )

                    for idx, (hl, b) in enumerate(flat):
                        if idx + 4 < len(flat):
                            load_pair(*flat[idx + 4], False)
                        pr = b * HPC + hl
                        np_b = npb[b]
                        jtk = np_b // P
                        ws = chunk_widths(np_b)
                        qt, kt, vt = tiles[(hl, b)]

                        if True:
                            off = 0
                            for ci, w in enumerate(ws):
                                o_t = psO.tile(
                                    [P, CW], f32, tag="ops",
                                    name=f"o{pr}_{ci}",
                                )
                                for gi, grp in enumerate(jt_groups(jtk)):
                                    s_t = psS.tile(
                                        [P, 2, CW], f32, tag="sps",
                                        name=f"s{pr}_{ci}_{gi}",
                                    )
                                    for slot, jt in enumerate(grp):
                                        nc.tensor.matmul(
                                            s_t[:, slot, :w],
                                            lhsT=kt[:, jt * P : (jt + 1) * P],
                                            rhs=qt[:, off : off + w],
                                            start=True,
                                            stop=True,
                                        )
                                    pt = ptp.tile(
                                        [P, 2, CW], bf16, tag="pt",
                                        name=f"p{pr}_{ci}_{gi}",
                                    )
                                    ng = len(grp)
                                    nc.scalar.activation(
                                        pt[:, 0:ng, :w],
                                        s_t[:, 0:ng, :w],
                                        mybir.ActivationFunctionType.Exp,
                                        scale=SCALE,
                                    )
                                    if len(pending) >= 2:
                                        flush_one()
                                    pending.append((o_t, vt, pt, w, grp, jtk))
                                    if grp[-1] == jtk - 1:
                                        evac_q.append((o_t, w, hl, G[b] + off))
                                off += w
                        if b == B - 1:
                            # exchange this head-half once its last chunk's
                            # PV + evacuation are flushed (a couple of steps
                            # into the next pair for hl=0)
                            cc_q = hl
                    while pending:
                        flush_one()

                    # bridge warm-ups: keep the PE clock ramped through the
                    # second A2A + gather window (psS-pool target so they
                    # don't WAR-stall on the final evacuation's PSUM reads)
                    def pin(mm, after, why):
                        tile.add_dep_helper(
                            mm.ins, after.ins, sync=False, reason=why
                        )
                        return mm

                with (
                    tc.tile_pool(name="outp", bufs=2) as outp,
                    tc.tile_pool(name="psP", bufs=2, space="PSUM") as psP,
                    tc.tile_pool(name="psW", bufs=1, space="PSUM") as psW,
                ):
                    wbA = psW.tile([P, 3, CW], f32, name="wbA")
                    def gather_half(h):
                        # pack src pairs (2cp, 2cp+1) into 128-deep tiles
                        for x in range(2):
                            for par in range(2):
                                nc.sync.dma_start(
                                    gat[h][
                                        par * D : (par + 1) * D,
                                        :,
                                        x * RBH : (x + 1) * RBH,
                                    ],
                                    a2a_out[h][x][par::2]
                                    .rearrange("c d l -> d c l"),
                                )

                    def proj_half(h, emit_out):
                        # interleave two row tiles' matmuls so consecutive
                        # PE instructions hit different PSUM banks
                        for rtp in range(0, NRT, 2):
                            rts = [rt for rt in (rtp, rtp + 1) if rt < NRT]
                            pt_ps = {
                                rt: psP.tile(
                                    [P, DIM], f32, tag="prps",
                                    name=f"pr{h}_{rt}",
                                )
                                for rt in rts
                            }
                            for cp in range(4):
                                for n0 in range(0, DIM, 512):
                                    for rt in rts:
                                        rows = min(P, RBq - rt * P)
                                        pin(
                                            nc.tensor.matmul(
                                                pt_ps[rt][:rows, n0 : n0 + 512],
                                                lhsT=gat[h][
                                                    :, cp, rt * P : rt * P + rows
                                                ],
                                                rhs=wt_sb[:, h, cp, n0 : n0 + 512],
                                                start=(cp == 0),
                                                stop=(cp == 3),
                                            ),
                                            last_pv,
                                            "projection follows attention",
                                        )
                            for rt in rts:
                                emit_out(rt, pt_ps[rt])

                    # hl=0's A2A completed mid-attention: its half of the
                    # projection runs DURING the second collective's window
                    gather_half(0)

                    def out0(rt, ps):
                        rows = min(P, RBq - rt * P)
                        nc.vector.tensor_copy(osb0[:rows, rt, :], ps[:rows, :])

                    proj_half(0, out0)

                    # bridge warm-ups: rotate targets across 3 PSUM banks so
                    # the drains pipeline
                    wtgts = [wbA[:, 0, :], wbA[:, 1, :], wbA[:, 2, :]]
                    last_warm = last_pv
                    for i in range(40):
                        last_warm = pin(
                            nc.tensor.matmul(
                                wtgts[i % 3],
                                lhsT=warmw[:, 0:P],
                                rhs=warmw[:],
                                start=True,
                                stop=True,
                                skip_group_check=True,
                            ),
                            last_pv,
                            "warmups bridge the A2A window",
                        )

                    gather_half(1)

                    def out1(rt, ps):
                        rows = min(P, RBq - rt * P)
                        osb = outp.tile([P, DIM], f32, tag="osb", name=f"ob{rt}")
                        nc.vector.tensor_tensor(
                            osb[:rows, :],
                            osb0[:rows, rt, :],
                            ps[:rows, :],
                            mybir.AluOpType.add,
                        )
                        nc.sync.dma_start(
                            outD[rt * P : rt * P + rows, :], osb[:rows, :]
                        )

                    proj_half(1, out1)

    nc.compile()
    return nc


def _get_nc(npb):
    key = f"nc{npb}"
    if key not in _CACHE:
        _CACHE[key] = build_graph(npb)
    return _CACHE[key]


def key_budget(mask):
    """Per-batch compacted row counts (unmasked incl. CLS), padded to 128."""
    counts = 1 + np.asarray(mask).astype(bool).sum(axis=1)
    return tuple(
        min(max(int(-(-int(c) // P) * P), P), N) for c in counts
    )


def make_in_maps(q, k, v, mask, W_out, b_out, npb):
    npmax = max(npb)
    q16 = np.asarray(q).astype(npbf)
    k16 = np.asarray(k).astype(npbf)
    v16 = np.asarray(v).astype(npbf)
    m_full = np.concatenate(
        [np.ones((B, 1), dtype=bool), np.asarray(mask).astype(bool)], axis=1
    )  # [B, N]

    qTall = np.zeros((B, H, D, npmax), dtype=npbf)
    kTall = np.zeros((B, H, D, npmax), dtype=npbf)
    vall = np.zeros((B, H, npmax, D + 1), dtype=npbf)
    for b in range(B):
        idx = np.flatnonzero(m_full[b])
        c = len(idx)
        qTall[b, :, :, :c] = q16[b][:, idx, :].transpose(0, 2, 1)
        kTall[b, :, :, :c] = k16[b][:, idx, :].transpose(0, 2, 1)
        vall[b, :, :c, :D] = v16[b][:, idx, :]
        vall[b, :, :c, D] = 1.0

    wT16 = np.ascontiguousarray(np.asarray(W_out).T).astype(npbf)
    # regroup W^T rows per head-half (see build_graph's wTs comment)
    wTs = np.empty((HPC, 4 * P, DIM), dtype=npbf)
    for hl in range(HPC):
        for cp in range(4):
            for par in range(2):
                h = 4 * cp + 2 * par + hl
                wTs[hl, cp * P + par * D : cp * P + (par + 1) * D] = wT16[
                    h * D : (h + 1) * D
                ]

    in_maps = []
    for c in range(NCORES):
        heads = slice(HPC * c, HPC * (c + 1))
        in_maps.append(
            {
                "qT": np.ascontiguousarray(
                    qTall[:, heads].reshape(PAIRS, D, npmax)
                ),
                "kT": np.ascontiguousarray(
                    kTall[:, heads].reshape(PAIRS, D, npmax)
                ),
                "v": np.ascontiguousarray(
                    vall[:, heads].reshape(PAIRS, npmax, D + 1)
                ),
                "wTs": wTs,
            }
        )
    return in_maps


def run(q, k, v, mask, W_out, b_out, trace=False, **spmd_kwargs):
    npb = key_budget(mask)
    nc = _get_nc(npb)
    in_maps = make_in_maps(q, k, v, mask, W_out, b_out, npb)
    res = run_bass_kernel_spmd(
        nc, in_maps, core_ids=list(range(NCORES)), trace=trace, **spmd_kwargs
    )
    proj = np.concatenate(
        [np.asarray(res.results[c]["out"]) for c in range(NCORES)], axis=0
    )  # [TQ, DIM]

    m_full = np.concatenate(
        [np.ones((B, 1), dtype=bool), np.asarray(mask).astype(bool)], axis=1
    )
    W32 = np.asarray(W_out, dtype=np.float32)
    b32 = np.asarray(b_out, dtype=np.float32)
    v32 = np.asarray(v, dtype=np.float32)
    full = np.empty((B, N, DIM), dtype=np.float32)
    g0 = 0
    for b in range(B):
        idx = np.flatnonzero(m_full[b])
        full[b, idx] = proj[g0 : g0 + len(idx)] + b32
        # masked queries: uniform attention over ALL N keys
        vmean = v32[b].transpose(1, 0, 2).reshape(N, DIM).mean(axis=0)
        full[b, ~m_full[b]] = vmean @ W32.T + b32
        g0 += npb[b]
    return full, res


def kernel(q, k, v, mask, W_out, b_out):
    out, _ = run(q, k, v, mask, W_out, b_out, trace=False)
    return out
